# revision 49
# baseline (speedup 1.0000x reference)
"""DMPNN encoder on 8 Trainium2 NeuronCores (Bass/Tile).

Strategy (data-parallel over graphs):
- Partition graphs into 8 contiguous chunks with ~equal atom counts
  (graph-aligned).  Each core owns the edges whose *target* atom lives in
  its chunk, sorted by local target -> segment-sum over targets is local.
- Per message-passing round, each core computes nei = segsum(msg) via
  selection-matrix matmuls, then Z = nei @ W_h on its own atoms, then the
  Z shards are AllGather'd across the 8 cores.  msg' = relu(msg + Z[src])
  only needs row gathers (indirect DMA) from the gathered Z table.
- Final round: atom_msg -> atom_h = relu([x||atom_msg] @ W_o + b_o) and
  sum-pool to graphs via selection matmuls; host sums partial group blocks.

All index manipulation is host-precomputed; the device does only dense
matmuls, elementwise ops, contiguous DMA and indirect row gathers.
"""

import os
import sys

for _p in ("/opt/trn_rl_repo", "/root/.axon_site/_ro/trn_rl_repo"):
    if os.path.isdir(_p) and _p not in sys.path:
        sys.path.insert(0, _p)

from contextlib import ExitStack

import numpy as np

import concourse.bass as bass
import concourse.tile as tile
from concourse import bacc, mybir
from concourse.bass_utils import run_bass_kernel_spmd
from concourse.masks import make_identity
from concourse.tile_rust import add_dep_helper

C = 8
H = 300
AF = 133
BF = 14
DEPTH = 3
NUM_GRAPHS = 4096

F32 = mybir.dt.float32
BF16 = mybir.dt.bfloat16
I32 = mybir.dt.int32

DRM = mybir.MatmulPerfMode.DoubleRow
Relu = mybir.ActivationFunctionType.Relu
Copy = mybir.ActivationFunctionType.Copy
ADD = mybir.AluOpType.add
ISEQ = mybir.AluOpType.is_equal
BYPASS = mybir.AluOpType.bypass

IOA = bass.IndirectOffsetOnAxis


def ceil_to(x, m):
    return ((x + m - 1) // m) * m


# ---------------------------------------------------------------------------
# host-side preprocessing
# ---------------------------------------------------------------------------

def preprocess(x, edge_attr, edge_index, batch, num_graphs):
    N = x.shape[0]
    src = edge_index[0].astype(np.int64)
    tgt = edge_index[1].astype(np.int64)
    batch = batch.astype(np.int64)

    graph_start = np.searchsorted(batch, np.arange(num_graphs + 1))
    targets = (np.arange(1, C) * N) // C
    split_graphs = np.searchsorted(graph_start, targets)
    atom_splits = [0] + [int(graph_start[g]) for g in split_graphs] + [N]
    a0 = np.array(atom_splits[:-1])
    a1 = np.array(atom_splits[1:])
    n_real = a1 - a0
    N_pad = ceil_to(int(n_real.max()), 128) + 128
    NA = N_pad // 128

    owner = np.zeros(N, dtype=np.int64)
    loc = np.zeros(N, dtype=np.int64)
    for c in range(C):
        owner[a0[c]:a1[c]] = c
        loc[a0[c]:a1[c]] = np.arange(n_real[c])

    e_owner = owner[tgt]

    # per-core, per-ablock sorted edge lists (slot order within an ablock is
    # irrelevant for the selection matmuls)
    ab_eids = []  # [c][a] -> array of edge ids
    counts = np.zeros((C, NA), dtype=np.int64)
    for c in range(C):
        eids = np.nonzero(e_owner == c)[0]
        order = np.argsort(loc[tgt[eids]], kind="stable")
        eids = eids[order]
        ab = loc[tgt[eids]] // 128
        cuts = np.searchsorted(ab, np.arange(NA + 1))
        ab_eids.append([eids[cuts[a]:cuts[a + 1]] for a in range(NA)])
        counts[c] = cuts[1:] - cuts[:-1]

    for GRP in (64, 32, 16, 8, 4, 2, 1):
        ok = True
        for c in range(C):
            nb = int(n_real[c])
            for g0a in range(0, NA, GRP):
                alo, ahi = g0a * 128, min((g0a + GRP) * 128, nb)
                if alo >= nb:
                    continue
                if batch[a0[c] + ahi - 1] - batch[a0[c] + alo] >= 128:
                    ok = False
                    break
            if not ok:
                break
        if ok:
            break
    NGRP = (NA + GRP - 1) // GRP

    # greedy segmentation over ablocks: keep the union live-depth of nei
    # accumulators <= DMAX so only DMAX PSUM slots are needed
    DMAX = 2

    def seg_stats(s, e):
        P = np.zeros((C,), dtype=np.int64)
        first = {}
        last = {}
        for a in range(s, e):
            cnt = counts[:, a]
            act = cnt > 0
            if act.any():
                first[a] = int((P[act] // 128).min())
                last[a] = int(((P[act] + cnt[act] - 1) // 128).max())
            P += cnt
        ntile = int(-(-P.max() // 128)) if P.max() else 1
        depth = 0
        for t in range(ntile):
            depth = max(depth, sum(1 for a in first
                                   if first[a] <= t <= last[a]))
        return first, last, ntile, depth

    segments = []  # (a_start, a_end, first, last, ntiles)
    s = 0
    while s < NA:
        e = s + 1
        stats = seg_stats(s, e)
        while e < NA and (e % GRP) != 0:
            cand = seg_stats(s, e + 1)
            if cand[3] > DMAX:
                break
            e += 1
            stats = cand
        segments.append((s, e, stats[0], stats[1], stats[2]))
        s = e

    T = sum(sg[4] for sg in segments)
    Epad = T * 128

    contribs = [[] for _ in range(T)]
    base = 0
    for (s, e, first, last, ntiles) in segments:
        for a in range(s, e):
            if a in first:
                ts = range(base + first[a], base + last[a] + 1)
            else:
                ts = [base]  # no edges anywhere: zero matmul, still drains
            ts = list(ts)
            for i, t in enumerate(ts):
                contribs[t].append((a, i == 0, i == len(ts) - 1))
        base += ntiles

    # contrib enumeration order + chunk boundaries for batched sel builds
    contrib_idx = {}
    ci = 0
    for t in range(T):
        for (a, f, l) in contribs[t]:
            contrib_idx[(t, a)] = ci
            ci += 1
    NCONTRIB = ci

    drain_seq = [a for t in range(T) for (a, f, l) in contribs[t] if l]
    pool_first = {}
    pool_last = {}
    seen = {}
    for a in drain_seq:
        j = a // GRP
        if j not in seen:
            pool_first[a] = True
        seen[j] = a
    for j, a in seen.items():
        pool_last[a] = True

    # collective chunking: chunk-major Z-table layout so each chunk's
    # AllGather output is a contiguous region.  Small chunks first (fire the
    # CC stream early) and last (shrink the un-overlappable tail), big in
    # the middle to amortize per-op overhead.
    frac = np.array([6, 28, 42, 34, 14, 6], dtype=np.float64)
    sizes = np.maximum(1, np.round(frac / frac.sum() * NA).astype(np.int64))
    cc_chunks = []
    a = 0
    for i, u in enumerate(sizes):
        e = NA if i == len(sizes) - 1 else min(a + int(u), NA)
        if e > a:
            cc_chunks.append((a, e))
        a = e
        if a >= NA:
            break
    n_cc = len(cc_chunks)
    cc_start = np.array([k0 * 128 for (k0, k1) in cc_chunks])
    cc_rows = np.array([(k1 - k0) * 128 for (k0, k1) in cc_chunks])
    cc_off = np.concatenate([[0], np.cumsum(C * cc_rows)])[:-1]

    # lay out slots: per core, segments concatenated, padded to common size
    per_core = []
    for c in range(C):
        slot_eid = np.full(Epad, -1, dtype=np.int64)
        base = 0
        for (s, e, first, last, ntiles) in segments:
            seg = np.concatenate([ab_eids[c][a] for a in range(s, e)]) \
                if e > s else np.zeros(0, np.int64)
            slot_eid[base:base + len(seg)] = seg
            base += ntiles * 128
        real = slot_eid >= 0
        eids = slot_eid[real]
        ne = int(real.sum())
        tloc = np.full(Epad, -1, dtype=np.int64)
        tloc[real] = loc[tgt[eids]]
        src_g = np.zeros(Epad, dtype=np.int64)
        src_g[real] = src[eids]
        ea = np.zeros((Epad, BF), dtype=np.float32)
        ea[real] = edge_attr[eids]
        zrow = np.zeros(Epad, dtype=np.int64)
        ol = owner[src_g[real]]
        ll = loc[src_g[real]]
        ck = np.searchsorted(cc_start, ll, side="right") - 1
        zrow[real] = cc_off[ck] + ol * cc_rows[ck] + (ll - cc_start[ck])
        uniq, inv = np.unique(src_g[real], return_inverse=True)
        xidx = np.zeros(Epad, dtype=np.int64)
        xidx[real] = inv
        xcat = np.zeros((Epad, AF + BF), dtype=np.float32)
        xcat[:, :AF] = x[src_g]
        xcat[:, AF:] = ea
        ctabm = np.zeros((128, NCONTRIB), dtype=np.float32)
        for t in range(T):
            for (a, f, l) in contribs[t]:
                i = contrib_idx[(t, a)]
                ctabm[:, i] = (tloc[t * 128:(t + 1) * 128] - 128 * a)
        selm = (ctabm[:, :, None] == np.arange(128, dtype=np.float32)
                ).astype(np.float32).reshape(128, NCONTRIB * 128)
        per_core.append(dict(ne=ne, real=real, tloc=tloc, ea=ea, zrow=zrow,
                             xidx=xidx, uniq=uniq, n_uniq=len(uniq),
                             selm=selm, xcat=xcat))

    X_pad = max(pc["n_uniq"] for pc in per_core) + 1
    for pc in per_core:
        pc["xidx"][~pc["real"]] = X_pad - 1
        xs = np.zeros((X_pad, AF), dtype=np.float32)
        xs[: pc["n_uniq"]] = x[pc["uniq"]]
        pc["x_sub"] = xs

    for c in range(C):
        gloc = np.full(N_pad, 1 << 20, dtype=np.int64)
        g0s = np.full(NGRP, -1, dtype=np.int64)
        nb = int(n_real[c])
        for j in range(NGRP):
            alo = j * GRP * 128
            ahi = min(alo + GRP * 128, nb)
            if alo >= nb:
                continue
            g0 = batch[a0[c] + alo]
            g0s[j] = g0
            gloc[alo:ahi] = batch[a0[c] + alo: a0[c] + ahi] - g0
        per_core[c]["g0s"] = g0s
        glocm = np.minimum(gloc, 1 << 20).reshape(NA, 128).T.astype(np.float32)
        per_core[c]["selp"] = (
            glocm[:, :, None] == np.arange(128, dtype=np.float32)
        ).astype(np.float32).reshape(128, NA * 128)
        xo = np.zeros((N_pad, AF), dtype=np.float32)
        xo[:nb] = x[a0[c]: a1[c]]
        per_core[c]["x_own"] = xo

    meta = dict(N_pad=N_pad, NA=NA, Epad=Epad, T=T, X_pad=X_pad, GRP=GRP,
                NGRP=NGRP, a0=a0, a1=a1, n_real=n_real, contribs=contribs,
                pool_first=pool_first, pool_last=pool_last,
                contrib_idx=contrib_idx, NCONTRIB=NCONTRIB,
                cc_chunks=cc_chunks, cc_off=cc_off, cc_rows=cc_rows)
    return per_core, meta


# ---------------------------------------------------------------------------
# the Bass program (identical for all 8 cores; data differs per core)
# ---------------------------------------------------------------------------

def build_program(meta, DT=F32, CH=8):
    T, NA, N_pad = meta["T"], meta["NA"], meta["N_pad"]
    Epad, X_pad = meta["Epad"], meta["X_pad"]
    GRP, NGRP = meta["GRP"], meta["NGRP"]
    contribs = meta["contribs"]

    nc = bacc.Bacc("TRN2", target_bir_lowering=False, debug=False,
                   num_devices=C)
    DTZ = mybir.dt.float8e4  # Z-exchange dtype (halves collective traffic)

    DTZ0 = mybir.dt.float8e4
    KDR = (AF + BF + 1) // 2  # 74: DoubleRow K-partitions for 147(+1 pad)
    xcdr = nc.dram_tensor("xcdr", [KDR, 2, Epad], DTZ0, kind="ExternalInput")
    # xh_in[p, a, :] = (x @ W_o[:AF] + b_o) for atom a*128+p (host GEMM)
    xh_in = nc.dram_tensor("xh_in", [128, NA, H], DT, kind="ExternalInput")
    selm_in = nc.dram_tensor("selm_in", [128, meta["NCONTRIB"] * 128], DT,
                             kind="ExternalInput")
    selp_in = nc.dram_tensor("selp_in", [128, NA * 128], DT,
                             kind="ExternalInput")
    zrow_in = nc.dram_tensor("zrow_in", [128, T], I32, kind="ExternalInput")

    wname_shapes = dict(
        widr=[KDR, 2, H],
        wh0=[128, H], wh1=[128, H], wh2=[H - 256, H],
        wom0=[128, H], wom1=[128, H], wom2=[H - 256, H],
    )
    wname_dts = dict(widr=DTZ0, wh0=DT, wh1=DT, wh2=DT,
                     wom0=DT, wom1=DT, wom2=DT)
    w_in = {k: nc.dram_tensor(k, s, wname_dts[k], kind="ExternalInput")
            for k, s in wname_shapes.items()}

    molp = nc.dram_tensor("molp", [NGRP * 128, H], F32, kind="ExternalOutput")

    zfull1 = nc.dram_tensor("zfull1", [C * N_pad, H], DTZ, addr_space="Shared")
    zfull2 = nc.dram_tensor("zfull2", [C * N_pad, H], DTZ, addr_space="Shared")
    cc_chunks = meta["cc_chunks"]

    HC = [(0, 128), (128, 256), (256, H)]  # hidden-dim K chunks

    with tile.TileContext(nc) as tc, ExitStack() as ctx:
        const = ctx.enter_context(tc.tile_pool(name="const", bufs=1))
        sb = ctx.enter_context(tc.tile_pool(name="sb", bufs=4))
        ps = ctx.enter_context(tc.tile_pool(name="ps", bufs=2, space="PSUM"))
        dram = ctx.enter_context(tc.tile_pool(name="dram", bufs=1,
                                              space="DRAM"))

        # ---- residents ----
        def cload(name, src, shape, dt):
            tl = const.tile(shape, dt, tag=name)
            nc.sync.dma_start(tl[:], src[:])
            return tl

        zrow_s = cload("zrow", zrow_in, [128, T], I32)
        w_s = {k: cload(k, w_in[k], wname_shapes[k], wname_dts[k])
               for k in w_in}
        identF = const.tile([128, 128], DT, tag="identF")
        make_identity(nc, identF[:])


        msg1 = dram.tile([128, T * H], DTZ, tag="msg1")
        msg2 = dram.tile([128, T * H], DTZ, tag="msg2")
        zsh1 = dram.tile([N_pad, H], DTZ, tag="zsh1")
        zsh2 = dram.tile([N_pad, H], DTZ, tag="zsh2")

        psum_nei = {}
        psum_pool = {}

        def transpose_chunk(src_ap, c0, c1, dst_ap):
            """PE-transpose one bf16 column chunk of src_ap into dst_ap."""
            w = c1 - c0
            tp = ps.tile([128, 128], DT, tag="pB", bufs=2)
            nc.tensor.transpose(tp[:w, :], src_ap[:, c0:c1], identF[:])
            if c0 == 128:
                nc.scalar.copy(dst_ap, tp[:w, :])
            else:
                nc.vector.tensor_copy(dst_ap, tp[:w, :])

        cc_state = {}

        cur_zfull = [None]

        # ---- software-pipelined drains -----------------------------------
        # Drains are 2-3 phase generators pumped once per subsequent tile so
        # the PE never sits behind an instruction whose input (a PSUM->SBUF
        # copy on vector/scalar) hasn't landed yet.
        drain_q = []

        def pump(flush=False):
            while True:
                for g in list(drain_q):
                    try:
                        next(g)
                    except StopIteration:
                        drain_q.remove(g)
                if not (flush and drain_q):
                    break

        def drain_z(a, zsh):
            """psum_nei[a] -> Z = nei @ W_h -> zsh rows of ablock a."""
            nei_sb = sb.tile([128, H], DT, tag="neisb")
            nc.vector.tensor_copy(nei_sb[:], psum_nei.pop(a)[:])

            def gen():
                trs = sb.tile([128, 3, 128], DT, tag="trs")
                for ci, (c0, c1) in enumerate(HC):
                    transpose_chunk(nei_sb, c0, c1, trs[:c1 - c0, ci, :])
                yield
                zps = ps.tile([128, H], F32, tag="pD", bufs=1)
                for ci, ((c0, c1), wt) in enumerate(
                        zip(HC, (w_s["wh0"], w_s["wh1"], w_s["wh2"]))):
                    nc.tensor.matmul(zps[:], lhsT=trs[:c1 - c0, ci, :],
                                     rhs=wt[:],
                                     start=(ci == 0), stop=(ci == len(HC) - 1))
                zsb = sb.tile([128, H], DTZ, tag="zsb")
                nc.scalar.copy(zsb[:], zps[:])
                nc.sync.dma_start(zsh[a * 128:(a + 1) * 128, :], zsb[:])
                fire_cc(a, zsh, cur_zfull[0])

            drain_q.append(gen())

        FB = 4  # drain_final prefetch group (ablocks per DMA)
        final_cache = {}

        def final_group(a):
            g = a // FB
            if g not in final_cache:
                alo, ahi = g * FB, min((g + 1) * FB, NA)
                w = (ahi - alo) * 128
                gsl = slice(alo * 128, ahi * 128)
                nab = ahi - alo
                xhg = sb.tile([128, nab, H], DT, tag="xhg",
                              padded_shape=[128, FB, H], bufs=2)
                nc.sync.dma_start(xhg[:], xh_in[:, alo:ahi, :])
                selpg = sb.tile([128, w], DT, tag="selp",
                                padded_shape=[128, FB * 128], bufs=2)
                nc.sync.dma_start(selpg[:], selp_in[:, gsl])
                final_cache.clear()
                final_cache[g] = (xhg, selpg)
            return final_cache[g]

        def drain_final(a):
            """psum_nei[a] = atom_msg -> atom_h -> pool into group psum."""
            am_sb = sb.tile([128, H], DT, tag="neisb")
            nc.vector.tensor_copy(am_sb[:], psum_nei.pop(a)[:])
            xhg, selpg = final_group(a)
            ao = a - (a // FB) * FB
            osl = slice(ao * 128, ao * 128 + 128)

            def gen():
                at = sb.tile([128, 3, 128], DT, tag="atrs")
                for ci, (c0, c1) in enumerate(HC):
                    transpose_chunk(am_sb, c0, c1, at[:c1 - c0, ci, :])
                yield
                hps = ps.tile([128, H], F32, tag="pA", bufs=3)
                parts = [(at[:, 0, :], w_s["wom0"]),
                         (at[:, 1, :], w_s["wom1"]),
                         (at[:H - 256, 2, :], w_s["wom2"])]
                for ci, (lh, wt) in enumerate(parts):
                    nc.tensor.matmul(hps[:], lhsT=lh, rhs=wt[:],
                                     start=(ci == 0),
                                     stop=(ci == len(parts) - 1))
                hsum = sb.tile([128, H], DT, tag="hrelu")
                nc.vector.tensor_tensor(hsum[:], hps[:], xhg[:, ao, :],
                                        op=ADD)
                hrelu = sb.tile([128, H], DT, tag="hrelu")
                nc.vector.tensor_scalar_max(hrelu[:], hsum[:], 0.0)
                yield
                j = a // GRP
                first = meta["pool_first"].get(a, False)
                last = meta["pool_last"].get(a, False)
                if first:
                    psum_pool[j] = ps.tile([128, H], F32, tag="pD",
                                           name=f"pool{j}", bufs=1)
                nc.tensor.matmul(psum_pool[j][:], lhsT=selpg[:, osl],
                                 rhs=hrelu[:], start=first, stop=last)
                if last:
                    mol_sb = sb.tile([128, H], F32, tag="molsb")
                    nc.vector.tensor_copy(mol_sb[:], psum_pool.pop(j)[:])
                    nc.sync.dma_start(molp[j * 128:(j + 1) * 128, :],
                                      mol_sb[:])

            drain_q.append(gen())

        contrib_idx = meta["contrib_idx"]
        max_ncc = max(sum(len(contribs[t]) for t in range(t0, min(t0 + CH, T)))
                      for t0 in range(0, T, CH))

        def build_sels(t0, k):
            """Stream the host-precomputed sel matrices for tiles [t0,t0+k)."""
            idxs = [contrib_idx[(t, a)] for t in range(t0, t0 + k)
                    for (a, f, l) in contribs[t]]
            if not idxs:
                return None, 0
            i0, ncc = idxs[0], len(idxs)
            assert idxs == list(range(i0, i0 + ncc))
            selc = sb.tile([128, ncc * 128], DT, tag="sel",
                           padded_shape=[128, max_ncc * 128], name="selc")
            nc.sync.dma_start(selc[:], selm_in[:, i0 * 128:(i0 + ncc) * 128])
            return selc, i0

        def segsum_contrib(msg_ap, t, rnd, zsh, selc, i0):
            for (a, first, last) in contribs[t]:
                q = contrib_idx[(t, a)] - i0
                sel_ap = selc[:, q * 128:(q + 1) * 128]
                if first:
                    psum_nei[a] = ps.tile([128, H], F32, tag="pC", name=f"nei{a}", bufs=2)
                nc.tensor.matmul(psum_nei[a][:], lhsT=sel_ap, rhs=msg_ap,
                                 start=first, stop=last)
                if last:
                    if rnd < DEPTH:
                        drain_z(a, zsh)
                    else:
                        drain_final(a)

        # ---- stage A: initial messages + round-1 segsum ----
        def fire_cc(a, zsh, zfull):
            """After ablock a's drain DMA: if it completes a cc chunk, fire
            that chunk's AllGather."""
            st = cc_state.setdefault(id(zsh), dict(done=set()))
            st["done"].add(a)
            for (k0, k1) in cc_chunks:
                if all(x in st["done"] for x in range(k0, k1)) \
                        and (k0, k1) not in st.get("fired", set()):
                    st.setdefault("fired", set()).add((k0, k1))
                    ci = cc_chunks.index((k0, k1))
                    off = int(meta["cc_off"][ci])
                    nrows = int(meta["cc_rows"][ci])
                    cc = nc.gpsimd.collective_compute(
                        "AllGather", BYPASS,
                        replica_groups=[list(range(C))],
                        ins=[zsh[k0 * 128:k1 * 128, :]],
                        outs=[zfull[off:off + C * nrows, :]])
                    st["last_cc"] = cc

        # ---- stage A: msg1 = relu([x[src]||ea] @ W_i) + round-1 segsum ----
        # host supplies the per-slot concat table transposed; two matmuls
        # accumulate in PSUM and relu reads PSUM directly.
        cur_zfull[0] = zfull1
        for t0 in range(0, T, CH):
            k = min(CH, T - t0)
            selc, i0c = build_sels(t0, k)
            csl = slice(t0 * 128, (t0 + k) * 128)
            xc = sb.tile([KDR, 2, k * 128], DTZ, tag="xc0", bufs=3)
            nc.sync.dma_start(xc[:], xcdr[:, :, csl])
            msg_sb = sb.tile([128, k * H], DT, tag="msg", bufs=3)
            # pass 1: one fp8 DoubleRow matmul per tile (weights x16, relu
            # descales by 1/16); relus alternate scalar/vector
            for j in range(k):
                jsl = slice(j * 128, (j + 1) * 128)
                mps = ps.tile([128, H], F32, tag="pA", bufs=3)
                nc.tensor.matmul(mps[:], lhsT=xc[:, :, jsl],
                                 rhs=w_s["widr"][:], start=True, stop=True,
                                 perf_mode=DRM)
                m_ap = msg_sb[:, j * H:(j + 1) * H]
                if j % 2 == 0:
                    nc.vector.tensor_scalar(m_ap, mps[:], 0.0625, 0.0,
                                            op0=mybir.AluOpType.mult,
                                            op1=mybir.AluOpType.max)
                else:
                    nc.scalar.activation(m_ap, mps[:], Relu, scale=0.0625)
            # pass 2: segsums; drain phases interleave one tile behind
            for j in range(k):
                pump()
                segsum_contrib(msg_sb[:, j * H:(j + 1) * H], t0 + j, 1,
                               zsh1, selc, i0c)
            msg8 = sb.tile([128, k * H], DTZ, tag="msg8", bufs=3)
            nc.vector.tensor_copy(msg8[:], msg_sb[:])
            nc.sync.dma_start(msg1[:, t0 * H:(t0 + k) * H], msg8[:])
        pump(flush=True)

        # ---- stages B (round 2) and C (round 3 + readout) ----
        def stage_mid(msg_in, msg_out, zfull, zsh, rnd, cc_prev):
            zflat = zfull[:]
            for t0 in range(0, T, CH):
                k = min(CH, T - t0)
                selc, i0c = build_sels(t0, k)
                ld = sb.tile([128, k * H], DTZ, tag="ld", bufs=3)
                nc.sync.dma_start(ld[:], msg_in[:, t0 * H:(t0 + k) * H])
                mrel = sb.tile([128, k * H], DT, tag="msg", bufs=3)
                zg = sb.tile([128, k * H], DTZ, tag="zg", bufs=8)
                msum = sb.tile([128, k * H], DT, tag="msum")
                # two half-group add+relu chains so the first segsums start
                # after 4 gathers instead of 8
                kh = (k + 1) // 2
                for j in range(k):
                    t = t0 + j
                    nc.gpsimd.indirect_dma_start(
                        out=zg[:, j * H:(j + 1) * H], out_offset=None,
                        in_=zflat,
                        in_offset=IOA(ap=zrow_s[:, t:t + 1], axis=0))
                    if j + 1 == kh or j + 1 == k:
                        hsl = slice((0 if j + 1 == kh else kh) * H,
                                    (j + 1) * H)
                        nc.vector.tensor_tensor(msum[:, hsl], ld[:, hsl],
                                                zg[:, hsl], op=ADD)
                        nc.scalar.activation(mrel[:, hsl], msum[:, hsl],
                                             Relu)
                for j in range(k):
                    pump()
                    segsum_contrib(mrel[:, j * H:(j + 1) * H], t0 + j, rnd,
                                   zsh, selc, i0c)
                if msg_out is not None:
                    msg8 = sb.tile([128, k * H], DTZ, tag="msg8", bufs=3)
                    nc.vector.tensor_copy(msg8[:], mrel[:])
                    nc.sync.dma_start(msg_out[:, t0 * H:(t0 + k) * H],
                                      msg8[:])
            pump(flush=True)

        cur_zfull[0] = zfull2
        stage_mid(msg1, msg2, zfull1, zsh2, 2, None)
        stage_mid(msg2, None, zfull2, None, 3, None)

    nc.compile()
    return nc


# ---------------------------------------------------------------------------
# per-core input maps + output assembly
# ---------------------------------------------------------------------------

def np_dt(DT):
    import ml_dtypes
    return np.dtype(ml_dtypes.bfloat16) if DT == BF16 else np.float32


def make_in_maps(per_core, meta, W_i, W_h, W_o, b_o, DT=F32):
    import ml_dtypes
    T, NA = meta["T"], meta["NA"]
    d = np_dt(DT)
    d8 = np.dtype(ml_dtypes.float8_e4m3)
    KDR = (AF + BF + 1) // 2
    wi_pad = np.concatenate(
        [16.0 * W_i, np.zeros((2 * KDR - (AF + BF), H), np.float32)], axis=0)
    weights = dict(
        wh0=W_h[:128], wh1=W_h[128:256], wh2=W_h[256:],
        wom0=W_o[AF:AF + 128], wom1=W_o[AF + 128:AF + 256],
        wom2=W_o[AF + 256:],
    )
    weights = {k: np.ascontiguousarray(v.astype(d)) for k, v in weights.items()}
    weights["widr"] = np.ascontiguousarray(
        wi_pad.reshape(2, KDR, H).transpose(1, 0, 2).astype(d8))
    maps = []
    for pc in per_core:
        m = dict(weights)
        xcp = np.concatenate(
            [pc["xcat"],
             np.zeros((pc["xcat"].shape[0], 2 * KDR - (AF + BF)), np.float32)],
            axis=1)  # [Epad, 2*KDR]
        m["xcdr"] = np.ascontiguousarray(
            xcp.T.reshape(2, KDR, -1).transpose(1, 0, 2).astype(d8))
        xh = pc["x_own"].astype(np.float32) @ W_o[:AF].astype(np.float32) \
            + b_o.astype(np.float32)[None, :]  # [N_pad, H]
        m["xh_in"] = np.ascontiguousarray(
            xh.reshape(NA, 128, H).transpose(1, 0, 2).astype(d))
        m["selm_in"] = np.ascontiguousarray(pc["selm"].astype(d))
        m["selp_in"] = np.ascontiguousarray(pc["selp"].astype(d))
        m["zrow_in"] = np.ascontiguousarray(
            pc["zrow"].reshape(T, 128).T.astype(np.int32))
        maps.append(m)
    return maps


def assemble_mol(mol_parts, per_core, meta, num_graphs):
    out = np.zeros((num_graphs, H), dtype=np.float32)
    for c in range(C):
        g0s = per_core[c]["g0s"]
        for j in range(meta["NGRP"]):
            g0 = int(g0s[j])
            if g0 < 0:
                continue
            hi = min(g0 + 128, num_graphs)
            out[g0:hi] += mol_parts[c][j * 128: j * 128 + (hi - g0)]
    return out


# ---------------------------------------------------------------------------
# entry point
# ---------------------------------------------------------------------------

_prog_cache = {}


def _ensure_ntff_hook():
    """Register the axon NTFF profiling hook if the image's antenv lacks
    the axon_hooks module (profiling plumbing only; unused when
    trace=False)."""
    try:
        from antenv.axon_hooks import get_axon_ntff_profile_hook  # noqa
        return
    except ImportError:
        pass
    import types
    import antenv
    from trn_agent_boot.trn_boot import _ntff_profile_via_ctypes
    mod = types.ModuleType("antenv.axon_hooks")
    _h = [None]
    mod.set_axon_ntff_profile_hook = lambda h: _h.__setitem__(0, h)
    mod.get_axon_ntff_profile_hook = lambda: _h[0]
    sys.modules["antenv.axon_hooks"] = mod
    antenv.axon_hooks = mod
    try:
        hook = _ntff_profile_via_ctypes("/opt/axon/libaxon_pjrt.so")
        if hook is not None:
            mod.set_axon_ntff_profile_hook(hook)
    except Exception:
        pass
    # artifact upload needs a bucket; irrelevant for local profiling
    import concourse.bass_utils as _bu
    _bu.upload_artifacts = lambda tmpdir: tmpdir


def _run(inputs, DT=F32, trace=False, tmpdir=None):
    per_core, meta = preprocess(
        inputs["x"], inputs["edge_attr"], inputs["edge_index"],
        inputs["batch"], NUM_GRAPHS)
    key = (meta["T"], meta["NA"], meta["X_pad"], meta["NGRP"], str(DT),
           str(np.asarray(inputs["edge_index"])[:, 0]))
    ck = (meta["T"], meta["NA"], meta["X_pad"], meta["NGRP"], str(DT))
    if ck not in _prog_cache:
        _prog_cache[ck] = build_program(meta, DT=DT)
    nc = _prog_cache[ck]
    in_maps = make_in_maps(per_core, meta, inputs["W_i"], inputs["W_h"],
                           inputs["W_o"], inputs["b_o"], DT=DT)
    if trace:
        _ensure_ntff_hook()
    res = run_bass_kernel_spmd(nc, in_maps, list(range(C)), trace=trace,
                               tmpdir=tmpdir)
    mol_parts = [res.results[c]["molp"].astype(np.float32) for c in range(C)]
    out = assemble_mol(mol_parts, per_core, meta, NUM_GRAPHS)
    return out, res


def kernel(x, edge_attr, W_i, W_h, W_o, b_o, edge_index, batch):
    inputs = dict(x=np.asarray(x), edge_attr=np.asarray(edge_attr),
                  W_i=np.asarray(W_i), W_h=np.asarray(W_h),
                  W_o=np.asarray(W_o), b_o=np.asarray(b_o),
                  edge_index=np.asarray(edge_index),
                  batch=np.asarray(batch))
    out, _ = _run(inputs, DT=BF16)
    return out



# revision 51
# speedup vs baseline: 1.1309x; 1.1309x over previous
"""DMPNN encoder on 8 Trainium2 NeuronCores (Bass/Tile).

Strategy (data-parallel over graphs):
- Partition graphs into 8 contiguous chunks with ~equal atom counts
  (graph-aligned).  Each core owns the edges whose *target* atom lives in
  its chunk, sorted by local target -> segment-sum over targets is local.
- Per message-passing round, each core computes nei = segsum(msg) via
  selection-matrix matmuls, then Z = nei @ W_h on its own atoms, then the
  Z shards are AllGather'd across the 8 cores.  msg' = relu(msg + Z[src])
  only needs row gathers (indirect DMA) from the gathered Z table.
- Final round: atom_msg -> atom_h = relu([x||atom_msg] @ W_o + b_o) and
  sum-pool to graphs via selection matmuls; host sums partial group blocks.

All index manipulation is host-precomputed; the device does only dense
matmuls, elementwise ops, contiguous DMA and indirect row gathers.
"""

import os
import sys

for _p in ("/opt/trn_rl_repo", "/root/.axon_site/_ro/trn_rl_repo"):
    if os.path.isdir(_p) and _p not in sys.path:
        sys.path.insert(0, _p)

from contextlib import ExitStack

import numpy as np

import concourse.bass as bass
import concourse.tile as tile
from concourse import bacc, mybir
from concourse.bass_utils import run_bass_kernel_spmd
from concourse.masks import make_identity
from concourse.tile_rust import add_dep_helper

C = 8
H = 300
AF = 133
BF = 14
DEPTH = 3
NUM_GRAPHS = 4096

F32 = mybir.dt.float32
BF16 = mybir.dt.bfloat16
I32 = mybir.dt.int32

DRM = mybir.MatmulPerfMode.DoubleRow
Relu = mybir.ActivationFunctionType.Relu
Copy = mybir.ActivationFunctionType.Copy
ADD = mybir.AluOpType.add
ISEQ = mybir.AluOpType.is_equal
BYPASS = mybir.AluOpType.bypass

IOA = bass.IndirectOffsetOnAxis


def ceil_to(x, m):
    return ((x + m - 1) // m) * m


# ---------------------------------------------------------------------------
# host-side preprocessing
# ---------------------------------------------------------------------------

def preprocess(x, edge_attr, edge_index, batch, num_graphs):
    N = x.shape[0]
    src = edge_index[0].astype(np.int64)
    tgt = edge_index[1].astype(np.int64)
    batch = batch.astype(np.int64)

    graph_start = np.searchsorted(batch, np.arange(num_graphs + 1))
    targets = (np.arange(1, C) * N) // C
    split_graphs = np.searchsorted(graph_start, targets)
    atom_splits = [0] + [int(graph_start[g]) for g in split_graphs] + [N]
    a0 = np.array(atom_splits[:-1])
    a1 = np.array(atom_splits[1:])
    n_real = a1 - a0
    N_pad = ceil_to(int(n_real.max()), 128) + 128
    NA = N_pad // 128

    owner = np.zeros(N, dtype=np.int64)
    loc = np.zeros(N, dtype=np.int64)
    for c in range(C):
        owner[a0[c]:a1[c]] = c
        loc[a0[c]:a1[c]] = np.arange(n_real[c])

    e_owner = owner[tgt]

    # per-core, per-ablock sorted edge lists (slot order within an ablock is
    # irrelevant for the selection matmuls)
    ab_eids = []  # [c][a] -> array of edge ids
    counts = np.zeros((C, NA), dtype=np.int64)
    for c in range(C):
        eids = np.nonzero(e_owner == c)[0]
        order = np.argsort(loc[tgt[eids]], kind="stable")
        eids = eids[order]
        ab = loc[tgt[eids]] // 128
        cuts = np.searchsorted(ab, np.arange(NA + 1))
        ab_eids.append([eids[cuts[a]:cuts[a + 1]] for a in range(NA)])
        counts[c] = cuts[1:] - cuts[:-1]

    for GRP in (64, 32, 16, 8, 4, 2, 1):
        ok = True
        for c in range(C):
            nb = int(n_real[c])
            for g0a in range(0, NA, GRP):
                alo, ahi = g0a * 128, min((g0a + GRP) * 128, nb)
                if alo >= nb:
                    continue
                if batch[a0[c] + ahi - 1] - batch[a0[c] + alo] >= 128:
                    ok = False
                    break
            if not ok:
                break
        if ok:
            break
    NGRP = (NA + GRP - 1) // GRP

    # greedy segmentation over ablocks: keep the union live-depth of nei
    # accumulators <= DMAX so only DMAX PSUM slots are needed
    DMAX = 2

    def seg_stats(s, e):
        P = np.zeros((C,), dtype=np.int64)
        first = {}
        last = {}
        for a in range(s, e):
            cnt = counts[:, a]
            act = cnt > 0
            if act.any():
                first[a] = int((P[act] // 128).min())
                last[a] = int(((P[act] + cnt[act] - 1) // 128).max())
            P += cnt
        ntile = int(-(-P.max() // 128)) if P.max() else 1
        depth = 0
        for t in range(ntile):
            depth = max(depth, sum(1 for a in first
                                   if first[a] <= t <= last[a]))
        return first, last, ntile, depth

    segments = []  # (a_start, a_end, first, last, ntiles)
    s = 0
    while s < NA:
        e = s + 1
        stats = seg_stats(s, e)
        while e < NA and (e % GRP) != 0:
            cand = seg_stats(s, e + 1)
            if cand[3] > DMAX:
                break
            e += 1
            stats = cand
        segments.append((s, e, stats[0], stats[1], stats[2]))
        s = e

    T = sum(sg[4] for sg in segments)
    Epad = T * 128

    contribs = [[] for _ in range(T)]
    base = 0
    for (s, e, first, last, ntiles) in segments:
        for a in range(s, e):
            if a in first:
                ts = range(base + first[a], base + last[a] + 1)
            else:
                ts = [base]  # no edges anywhere: zero matmul, still drains
            ts = list(ts)
            for i, t in enumerate(ts):
                contribs[t].append((a, i == 0, i == len(ts) - 1))
        base += ntiles

    # contrib enumeration order + chunk boundaries for batched sel builds
    contrib_idx = {}
    ci = 0
    for t in range(T):
        for (a, f, l) in contribs[t]:
            contrib_idx[(t, a)] = ci
            ci += 1
    NCONTRIB = ci

    drain_seq = [a for t in range(T) for (a, f, l) in contribs[t] if l]
    pool_first = {}
    pool_last = {}
    seen = {}
    for a in drain_seq:
        j = a // GRP
        if j not in seen:
            pool_first[a] = True
        seen[j] = a
    for j, a in seen.items():
        pool_last[a] = True

    # collective chunking: chunk-major Z-table layout so each chunk's
    # AllGather output is a contiguous region.  Small chunks first (fire the
    # CC stream early) and last (shrink the un-overlappable tail), big in
    # the middle to amortize per-op overhead.
    frac = np.array([3, 6, 12, 18, 25, 18, 9, 5, 3, 2, 2], dtype=np.float64)
    sizes = np.maximum(1, np.round(frac / frac.sum() * NA).astype(np.int64))
    cc_chunks = []
    a = 0
    for i, u in enumerate(sizes):
        e = NA if i == len(sizes) - 1 else min(a + int(u), NA)
        if e > a:
            cc_chunks.append((a, e))
        a = e
        if a >= NA:
            break
    n_cc = len(cc_chunks)
    cc_start = np.array([k0 * 128 for (k0, k1) in cc_chunks])
    cc_rows = np.array([(k1 - k0) * 128 for (k0, k1) in cc_chunks])
    cc_off = np.concatenate([[0], np.cumsum(C * cc_rows)])[:-1]

    # lay out slots: per core, segments concatenated, padded to common size
    per_core = []
    for c in range(C):
        slot_eid = np.full(Epad, -1, dtype=np.int64)
        base = 0
        for (s, e, first, last, ntiles) in segments:
            seg = np.concatenate([ab_eids[c][a] for a in range(s, e)]) \
                if e > s else np.zeros(0, np.int64)
            slot_eid[base:base + len(seg)] = seg
            base += ntiles * 128
        real = slot_eid >= 0
        eids = slot_eid[real]
        ne = int(real.sum())
        tloc = np.full(Epad, -1, dtype=np.int64)
        tloc[real] = loc[tgt[eids]]
        src_g = np.zeros(Epad, dtype=np.int64)
        src_g[real] = src[eids]
        ea = np.zeros((Epad, BF), dtype=np.float32)
        ea[real] = edge_attr[eids]
        zrow = np.zeros(Epad, dtype=np.int64)
        ol = owner[src_g[real]]
        ll = loc[src_g[real]]
        ck = np.searchsorted(cc_start, ll, side="right") - 1
        zrow[real] = cc_off[ck] + ol * cc_rows[ck] + (ll - cc_start[ck])
        uniq, inv = np.unique(src_g[real], return_inverse=True)
        xidx = np.zeros(Epad, dtype=np.int64)
        xidx[real] = inv
        xcat = np.zeros((Epad, AF + BF), dtype=np.float32)
        xcat[:, :AF] = x[src_g]
        xcat[:, AF:] = ea
        ctabm = np.zeros((128, NCONTRIB), dtype=np.float32)
        for t in range(T):
            for (a, f, l) in contribs[t]:
                i = contrib_idx[(t, a)]
                ctabm[:, i] = (tloc[t * 128:(t + 1) * 128] - 128 * a)
        selm = (ctabm[:, :, None] == np.arange(128, dtype=np.float32)
                ).astype(np.float32).reshape(128, NCONTRIB * 128)
        per_core.append(dict(ne=ne, real=real, tloc=tloc, ea=ea, zrow=zrow,
                             xidx=xidx, uniq=uniq, n_uniq=len(uniq),
                             selm=selm, xcat=xcat))

    X_pad = max(pc["n_uniq"] for pc in per_core) + 1
    for pc in per_core:
        pc["xidx"][~pc["real"]] = X_pad - 1
        xs = np.zeros((X_pad, AF), dtype=np.float32)
        xs[: pc["n_uniq"]] = x[pc["uniq"]]
        pc["x_sub"] = xs

    for c in range(C):
        gloc = np.full(N_pad, 1 << 20, dtype=np.int64)
        g0s = np.full(NGRP, -1, dtype=np.int64)
        nb = int(n_real[c])
        for j in range(NGRP):
            alo = j * GRP * 128
            ahi = min(alo + GRP * 128, nb)
            if alo >= nb:
                continue
            g0 = batch[a0[c] + alo]
            g0s[j] = g0
            gloc[alo:ahi] = batch[a0[c] + alo: a0[c] + ahi] - g0
        per_core[c]["g0s"] = g0s
        glocm = np.minimum(gloc, 1 << 20).reshape(NA, 128).T.astype(np.float32)
        per_core[c]["selp"] = (
            glocm[:, :, None] == np.arange(128, dtype=np.float32)
        ).astype(np.float32).reshape(128, NA * 128)
        xo = np.zeros((N_pad, AF), dtype=np.float32)
        xo[:nb] = x[a0[c]: a1[c]]
        per_core[c]["x_own"] = xo

    meta = dict(N_pad=N_pad, NA=NA, Epad=Epad, T=T, X_pad=X_pad, GRP=GRP,
                NGRP=NGRP, a0=a0, a1=a1, n_real=n_real, contribs=contribs,
                pool_first=pool_first, pool_last=pool_last,
                contrib_idx=contrib_idx, NCONTRIB=NCONTRIB,
                cc_chunks=cc_chunks, cc_off=cc_off, cc_rows=cc_rows)
    return per_core, meta


# ---------------------------------------------------------------------------
# the Bass program (identical for all 8 cores; data differs per core)
# ---------------------------------------------------------------------------

def build_program(meta, DT=F32, CH=8):
    T, NA, N_pad = meta["T"], meta["NA"], meta["N_pad"]
    Epad, X_pad = meta["Epad"], meta["X_pad"]
    GRP, NGRP = meta["GRP"], meta["NGRP"]
    contribs = meta["contribs"]

    nc = bacc.Bacc("TRN2", target_bir_lowering=False, debug=False,
                   num_devices=C)
    DTZ = mybir.dt.float8e4  # Z-exchange dtype (halves collective traffic)

    DTZ0 = mybir.dt.float8e4
    KDR = (AF + BF + 1) // 2  # 74: DoubleRow K-partitions for 147(+1 pad)
    xcdr = nc.dram_tensor("xcdr", [KDR, 2, Epad], DTZ0, kind="ExternalInput")
    # xh_in[p, a, :] = (x @ W_o[:AF] + b_o) for atom a*128+p (host GEMM)
    xh_in = nc.dram_tensor("xh_in", [128, NA, H], DT, kind="ExternalInput")
    selm_in = nc.dram_tensor("selm_in", [128, meta["NCONTRIB"] * 128], DT,
                             kind="ExternalInput")
    selp_in = nc.dram_tensor("selp_in", [128, NA * 128], DT,
                             kind="ExternalInput")
    zrow_in = nc.dram_tensor("zrow_in", [128, T], I32, kind="ExternalInput")

    wname_shapes = dict(
        widr=[KDR, 2, H],
        wh0=[128, H], wh1=[128, H], wh2=[H - 256, H],
        wom0=[128, H], wom1=[128, H], wom2=[H - 256, H],
    )
    wname_dts = dict(widr=DTZ0, wh0=DT, wh1=DT, wh2=DT,
                     wom0=DT, wom1=DT, wom2=DT)
    w_in = {k: nc.dram_tensor(k, s, wname_dts[k], kind="ExternalInput")
            for k, s in wname_shapes.items()}

    molp = nc.dram_tensor("molp", [NGRP * 128, H], F32, kind="ExternalOutput")

    zfull1 = nc.dram_tensor("zfull1", [C * N_pad, H], DTZ, addr_space="Shared")
    zfull2 = nc.dram_tensor("zfull2", [C * N_pad, H], DTZ, addr_space="Shared")
    cc_chunks = meta["cc_chunks"]

    HC = [(0, 128), (128, 256), (256, H)]  # hidden-dim K chunks

    with tile.TileContext(nc) as tc, ExitStack() as ctx:
        const = ctx.enter_context(tc.tile_pool(name="const", bufs=1))
        sb = ctx.enter_context(tc.tile_pool(name="sb", bufs=4))
        ps = ctx.enter_context(tc.tile_pool(name="ps", bufs=2, space="PSUM"))
        dram = ctx.enter_context(tc.tile_pool(name="dram", bufs=1,
                                              space="DRAM"))

        # ---- residents ----
        def cload(name, src, shape, dt):
            tl = const.tile(shape, dt, tag=name)
            nc.sync.dma_start(tl[:], src[:])
            return tl

        zrow_s = cload("zrow", zrow_in, [128, T], I32)
        w_s = {k: cload(k, w_in[k], wname_shapes[k], wname_dts[k])
               for k in w_in}
        identF = const.tile([128, 128], DT, tag="identF")
        make_identity(nc, identF[:])


        msg1 = dram.tile([128, T * H], DTZ, tag="msg1")
        msg2 = dram.tile([128, T * H], DTZ, tag="msg2")
        zsh1 = dram.tile([N_pad, H], DTZ, tag="zsh1")
        zsh2 = dram.tile([N_pad, H], DTZ, tag="zsh2")

        psum_nei = {}
        psum_pool = {}

        def transpose_chunk(src_ap, c0, c1, dst_ap):
            """PE-transpose one bf16 column chunk of src_ap into dst_ap."""
            w = c1 - c0
            tp = ps.tile([128, 128], DT, tag="pB", bufs=2)
            nc.tensor.transpose(tp[:w, :], src_ap[:, c0:c1], identF[:])
            if c0 == 128:
                nc.scalar.copy(dst_ap, tp[:w, :])
            else:
                nc.vector.tensor_copy(dst_ap, tp[:w, :])

        cc_state = {}

        cur_zfull = [None]

        # ---- software-pipelined drains -----------------------------------
        # Drains are 2-3 phase generators pumped once per subsequent tile so
        # the PE never sits behind an instruction whose input (a PSUM->SBUF
        # copy on vector/scalar) hasn't landed yet.
        drain_q = []

        def pump(flush=False):
            while True:
                for g in list(drain_q):
                    try:
                        next(g)
                    except StopIteration:
                        drain_q.remove(g)
                if not (flush and drain_q):
                    break

        def drain_z(a, zsh):
            """psum_nei[a] -> Z = nei @ W_h -> zsh rows of ablock a."""
            nei_sb = sb.tile([128, H], DT, tag="neisb")
            nc.vector.tensor_copy(nei_sb[:], psum_nei.pop(a)[:])

            def gen():
                trs = sb.tile([128, 3, 128], DT, tag="trs")
                for ci, (c0, c1) in enumerate(HC):
                    transpose_chunk(nei_sb, c0, c1, trs[:c1 - c0, ci, :])
                yield
                zps = ps.tile([128, H], F32, tag="pD", bufs=1)
                for ci, ((c0, c1), wt) in enumerate(
                        zip(HC, (w_s["wh0"], w_s["wh1"], w_s["wh2"]))):
                    nc.tensor.matmul(zps[:], lhsT=trs[:c1 - c0, ci, :],
                                     rhs=wt[:],
                                     start=(ci == 0), stop=(ci == len(HC) - 1))
                zsb = sb.tile([128, H], DTZ, tag="zsb")
                nc.scalar.copy(zsb[:], zps[:])
                nc.sync.dma_start(zsh[a * 128:(a + 1) * 128, :], zsb[:])
                fire_cc(a, zsh, cur_zfull[0])

            drain_q.append(gen())

        FB = 4  # drain_final prefetch group (ablocks per DMA)
        final_cache = {}

        def final_group(a):
            g = a // FB
            if g not in final_cache:
                alo, ahi = g * FB, min((g + 1) * FB, NA)
                w = (ahi - alo) * 128
                gsl = slice(alo * 128, ahi * 128)
                nab = ahi - alo
                xhg = sb.tile([128, nab, H], DT, tag="xhg",
                              padded_shape=[128, FB, H], bufs=2)
                nc.sync.dma_start(xhg[:], xh_in[:, alo:ahi, :])
                selpg = sb.tile([128, w], DT, tag="selp",
                                padded_shape=[128, FB * 128], bufs=2)
                nc.sync.dma_start(selpg[:], selp_in[:, gsl])
                final_cache.clear()
                final_cache[g] = (xhg, selpg)
            return final_cache[g]

        def drain_final(a):
            """psum_nei[a] = atom_msg -> atom_h -> pool into group psum."""
            am_sb = sb.tile([128, H], DT, tag="neisb")
            nc.vector.tensor_copy(am_sb[:], psum_nei.pop(a)[:])
            xhg, selpg = final_group(a)
            ao = a - (a // FB) * FB
            osl = slice(ao * 128, ao * 128 + 128)

            def gen():
                at = sb.tile([128, 3, 128], DT, tag="atrs")
                for ci, (c0, c1) in enumerate(HC):
                    transpose_chunk(am_sb, c0, c1, at[:c1 - c0, ci, :])
                yield
                hps = ps.tile([128, H], F32, tag="pA", bufs=3)
                parts = [(at[:, 0, :], w_s["wom0"]),
                         (at[:, 1, :], w_s["wom1"]),
                         (at[:H - 256, 2, :], w_s["wom2"])]
                for ci, (lh, wt) in enumerate(parts):
                    nc.tensor.matmul(hps[:], lhsT=lh, rhs=wt[:],
                                     start=(ci == 0),
                                     stop=(ci == len(parts) - 1))
                hsum = sb.tile([128, H], DT, tag="hrelu")
                nc.vector.tensor_tensor(hsum[:], hps[:], xhg[:, ao, :],
                                        op=ADD)
                hrelu = sb.tile([128, H], DT, tag="hrelu")
                nc.vector.tensor_scalar_max(hrelu[:], hsum[:], 0.0)
                yield
                j = a // GRP
                first = meta["pool_first"].get(a, False)
                last = meta["pool_last"].get(a, False)
                if first:
                    psum_pool[j] = ps.tile([128, H], F32, tag="pD",
                                           name=f"pool{j}", bufs=1)
                nc.tensor.matmul(psum_pool[j][:], lhsT=selpg[:, osl],
                                 rhs=hrelu[:], start=first, stop=last)
                if last:
                    mol_sb = sb.tile([128, H], F32, tag="molsb")
                    nc.vector.tensor_copy(mol_sb[:], psum_pool.pop(j)[:])
                    nc.sync.dma_start(molp[j * 128:(j + 1) * 128, :],
                                      mol_sb[:])

            drain_q.append(gen())

        contrib_idx = meta["contrib_idx"]
        max_ncc = max(sum(len(contribs[t]) for t in range(t0, min(t0 + CH, T)))
                      for t0 in range(0, T, CH))

        def build_sels(t0, k):
            """Stream the host-precomputed sel matrices for tiles [t0,t0+k)."""
            idxs = [contrib_idx[(t, a)] for t in range(t0, t0 + k)
                    for (a, f, l) in contribs[t]]
            if not idxs:
                return None, 0
            i0, ncc = idxs[0], len(idxs)
            assert idxs == list(range(i0, i0 + ncc))
            selc = sb.tile([128, ncc * 128], DT, tag="sel",
                           padded_shape=[128, max_ncc * 128], name="selc")
            nc.sync.dma_start(selc[:], selm_in[:, i0 * 128:(i0 + ncc) * 128])
            return selc, i0

        def segsum_contrib(msg_ap, t, rnd, zsh, selc, i0):
            for (a, first, last) in contribs[t]:
                q = contrib_idx[(t, a)] - i0
                sel_ap = selc[:, q * 128:(q + 1) * 128]
                if first:
                    psum_nei[a] = ps.tile([128, H], F32, tag="pC", name=f"nei{a}", bufs=2)
                nc.tensor.matmul(psum_nei[a][:], lhsT=sel_ap, rhs=msg_ap,
                                 start=first, stop=last)
                if last:
                    if rnd < DEPTH:
                        drain_z(a, zsh)
                    else:
                        drain_final(a)

        # ---- stage A: initial messages + round-1 segsum ----
        def fire_cc(a, zsh, zfull):
            """After ablock a's drain DMA: if it completes a cc chunk, fire
            that chunk's AllGather."""
            st = cc_state.setdefault(id(zsh), dict(done=set()))
            st["done"].add(a)
            for (k0, k1) in cc_chunks:
                if all(x in st["done"] for x in range(k0, k1)) \
                        and (k0, k1) not in st.get("fired", set()):
                    st.setdefault("fired", set()).add((k0, k1))
                    ci = cc_chunks.index((k0, k1))
                    off = int(meta["cc_off"][ci])
                    nrows = int(meta["cc_rows"][ci])
                    cc = nc.gpsimd.collective_compute(
                        "AllGather", BYPASS,
                        replica_groups=[list(range(C))],
                        ins=[zsh[k0 * 128:k1 * 128, :]],
                        outs=[zfull[off:off + C * nrows, :]])
                    st["last_cc"] = cc

        # ---- stage A: msg1 = relu([x[src]||ea] @ W_i) + round-1 segsum ----
        # host supplies the per-slot concat table transposed; two matmuls
        # accumulate in PSUM and relu reads PSUM directly.
        cur_zfull[0] = zfull1
        for t0 in range(0, T, CH):
            k = min(CH, T - t0)
            selc, i0c = build_sels(t0, k)
            csl = slice(t0 * 128, (t0 + k) * 128)
            xc = sb.tile([KDR, 2, k * 128], DTZ, tag="xc0", bufs=3)
            nc.sync.dma_start(xc[:], xcdr[:, :, csl])
            msg_sb = sb.tile([128, k * H], DT, tag="msg", bufs=3)
            # pass 1: one fp8 DoubleRow matmul per tile (weights x16, relu
            # descales by 1/16); relus alternate scalar/vector
            for j in range(k):
                jsl = slice(j * 128, (j + 1) * 128)
                mps = ps.tile([128, H], F32, tag="pA", bufs=3)
                nc.tensor.matmul(mps[:], lhsT=xc[:, :, jsl],
                                 rhs=w_s["widr"][:], start=True, stop=True,
                                 perf_mode=DRM)
                m_ap = msg_sb[:, j * H:(j + 1) * H]
                if j % 2 == 0:
                    nc.vector.tensor_scalar(m_ap, mps[:], 0.0625, 0.0,
                                            op0=mybir.AluOpType.mult,
                                            op1=mybir.AluOpType.max)
                else:
                    nc.scalar.activation(m_ap, mps[:], Relu, scale=0.0625)
            # pass 2: segsums; drain phases interleave one tile behind
            for j in range(k):
                pump()
                segsum_contrib(msg_sb[:, j * H:(j + 1) * H], t0 + j, 1,
                               zsh1, selc, i0c)
            msg8 = sb.tile([128, k * H], DTZ, tag="msg8", bufs=3)
            nc.vector.tensor_copy(msg8[:], msg_sb[:])
            nc.sync.dma_start(msg1[:, t0 * H:(t0 + k) * H], msg8[:])
        pump(flush=True)

        # ---- stages B (round 2) and C (round 3 + readout) ----
        def stage_mid(msg_in, msg_out, zfull, zsh, rnd, cc_prev):
            zflat = zfull[:]
            for t0 in range(0, T, CH):
                k = min(CH, T - t0)
                selc, i0c = build_sels(t0, k)
                ld = sb.tile([128, k * H], DTZ, tag="ld", bufs=3)
                nc.sync.dma_start(ld[:], msg_in[:, t0 * H:(t0 + k) * H])
                mrel = sb.tile([128, k * H], DT, tag="msg", bufs=3)
                zg = sb.tile([128, k * H], DTZ, tag="zg", bufs=6)
                msum = sb.tile([128, k * H], DT, tag="msum")
                # two half-group add+relu chains so the first segsums start
                # after 4 gathers instead of 8
                kh = (k + 1) // 2
                for j in range(k):
                    t = t0 + j
                    nc.gpsimd.indirect_dma_start(
                        out=zg[:, j * H:(j + 1) * H], out_offset=None,
                        in_=zflat,
                        in_offset=IOA(ap=zrow_s[:, t:t + 1], axis=0))
                    if j + 1 == kh or j + 1 == k:
                        hsl = slice((0 if j + 1 == kh else kh) * H,
                                    (j + 1) * H)
                        nc.vector.tensor_tensor(msum[:, hsl], ld[:, hsl],
                                                zg[:, hsl], op=ADD)
                        nc.scalar.activation(mrel[:, hsl], msum[:, hsl],
                                             Relu)
                for j in range(k):
                    pump()
                    segsum_contrib(mrel[:, j * H:(j + 1) * H], t0 + j, rnd,
                                   zsh, selc, i0c)
                if msg_out is not None:
                    msg8 = sb.tile([128, k * H], DTZ, tag="msg8", bufs=3)
                    nc.vector.tensor_copy(msg8[:], mrel[:])
                    nc.sync.dma_start(msg_out[:, t0 * H:(t0 + k) * H],
                                      msg8[:])
            pump(flush=True)

        cur_zfull[0] = zfull2
        stage_mid(msg1, msg2, zfull1, zsh2, 2, None)
        stage_mid(msg2, None, zfull2, None, 3, None)

    nc.compile()
    return nc


# ---------------------------------------------------------------------------
# per-core input maps + output assembly
# ---------------------------------------------------------------------------

def np_dt(DT):
    import ml_dtypes
    return np.dtype(ml_dtypes.bfloat16) if DT == BF16 else np.float32


def make_in_maps(per_core, meta, W_i, W_h, W_o, b_o, DT=F32):
    import ml_dtypes
    T, NA = meta["T"], meta["NA"]
    d = np_dt(DT)
    d8 = np.dtype(ml_dtypes.float8_e4m3)
    KDR = (AF + BF + 1) // 2
    wi_pad = np.concatenate(
        [16.0 * W_i, np.zeros((2 * KDR - (AF + BF), H), np.float32)], axis=0)
    weights = dict(
        wh0=W_h[:128], wh1=W_h[128:256], wh2=W_h[256:],
        wom0=W_o[AF:AF + 128], wom1=W_o[AF + 128:AF + 256],
        wom2=W_o[AF + 256:],
    )
    weights = {k: np.ascontiguousarray(v.astype(d)) for k, v in weights.items()}
    weights["widr"] = np.ascontiguousarray(
        wi_pad.reshape(2, KDR, H).transpose(1, 0, 2).astype(d8))
    maps = []
    for pc in per_core:
        m = dict(weights)
        xcp = np.concatenate(
            [pc["xcat"],
             np.zeros((pc["xcat"].shape[0], 2 * KDR - (AF + BF)), np.float32)],
            axis=1)  # [Epad, 2*KDR]
        m["xcdr"] = np.ascontiguousarray(
            xcp.T.reshape(2, KDR, -1).transpose(1, 0, 2).astype(d8))
        xh = pc["x_own"].astype(np.float32) @ W_o[:AF].astype(np.float32) \
            + b_o.astype(np.float32)[None, :]  # [N_pad, H]
        m["xh_in"] = np.ascontiguousarray(
            xh.reshape(NA, 128, H).transpose(1, 0, 2).astype(d))
        m["selm_in"] = np.ascontiguousarray(pc["selm"].astype(d))
        m["selp_in"] = np.ascontiguousarray(pc["selp"].astype(d))
        m["zrow_in"] = np.ascontiguousarray(
            pc["zrow"].reshape(T, 128).T.astype(np.int32))
        maps.append(m)
    return maps


def assemble_mol(mol_parts, per_core, meta, num_graphs):
    out = np.zeros((num_graphs, H), dtype=np.float32)
    for c in range(C):
        g0s = per_core[c]["g0s"]
        for j in range(meta["NGRP"]):
            g0 = int(g0s[j])
            if g0 < 0:
                continue
            hi = min(g0 + 128, num_graphs)
            out[g0:hi] += mol_parts[c][j * 128: j * 128 + (hi - g0)]
    return out


# ---------------------------------------------------------------------------
# entry point
# ---------------------------------------------------------------------------

_prog_cache = {}


def _ensure_ntff_hook():
    """Register the axon NTFF profiling hook if the image's antenv lacks
    the axon_hooks module (profiling plumbing only; unused when
    trace=False)."""
    try:
        from antenv.axon_hooks import get_axon_ntff_profile_hook  # noqa
        return
    except ImportError:
        pass
    import types
    import antenv
    from trn_agent_boot.trn_boot import _ntff_profile_via_ctypes
    mod = types.ModuleType("antenv.axon_hooks")
    _h = [None]
    mod.set_axon_ntff_profile_hook = lambda h: _h.__setitem__(0, h)
    mod.get_axon_ntff_profile_hook = lambda: _h[0]
    sys.modules["antenv.axon_hooks"] = mod
    antenv.axon_hooks = mod
    try:
        hook = _ntff_profile_via_ctypes("/opt/axon/libaxon_pjrt.so")
        if hook is not None:
            mod.set_axon_ntff_profile_hook(hook)
    except Exception:
        pass
    # artifact upload needs a bucket; irrelevant for local profiling
    import concourse.bass_utils as _bu
    _bu.upload_artifacts = lambda tmpdir: tmpdir


def _run(inputs, DT=F32, trace=False, tmpdir=None):
    per_core, meta = preprocess(
        inputs["x"], inputs["edge_attr"], inputs["edge_index"],
        inputs["batch"], NUM_GRAPHS)
    key = (meta["T"], meta["NA"], meta["X_pad"], meta["NGRP"], str(DT),
           str(np.asarray(inputs["edge_index"])[:, 0]))
    ck = (meta["T"], meta["NA"], meta["X_pad"], meta["NGRP"], str(DT))
    if ck not in _prog_cache:
        _prog_cache[ck] = build_program(meta, DT=DT)
    nc = _prog_cache[ck]
    in_maps = make_in_maps(per_core, meta, inputs["W_i"], inputs["W_h"],
                           inputs["W_o"], inputs["b_o"], DT=DT)
    if trace:
        _ensure_ntff_hook()
    res = run_bass_kernel_spmd(nc, in_maps, list(range(C)), trace=trace,
                               tmpdir=tmpdir)
    mol_parts = [res.results[c]["molp"].astype(np.float32) for c in range(C)]
    out = assemble_mol(mol_parts, per_core, meta, NUM_GRAPHS)
    return out, res


def kernel(x, edge_attr, W_i, W_h, W_o, b_o, edge_index, batch):
    inputs = dict(x=np.asarray(x), edge_attr=np.asarray(edge_attr),
                  W_i=np.asarray(W_i), W_h=np.asarray(W_h),
                  W_o=np.asarray(W_o), b_o=np.asarray(b_o),
                  edge_index=np.asarray(edge_index),
                  batch=np.asarray(batch))
    out, _ = _run(inputs, DT=BF16)
    return out

